# revision 1
# baseline (speedup 1.0000x reference)
"""Dynamic lightweight convolution TRN2 kernel.

out[b,l,d] = (1/K) * sum_k softmax_k(x[b,l+K-1,:] @ W + bias)[k, d%H] * x[b,l+k,d]

B=8, S=2048, D=1024, K=7, H=16, L=S-K+1=2042.
Sharding: data-parallel over batch, one batch element per NeuronCore (8 cores).

Per-core plan (channels on partitions; sequence on the free axis so the K=7
window shifts are free-axis offsets, which the compute engines allow):
  1. One DMA per 512-row block of x; PE-transpose 128x128 tiles; ScalarE
     copies PSUM->SBUF casting to bf16 -> xtb[d, s].
  2. logits = W^T @ xT on PE (bf16, fp32 PSUM accumulation over 8 d-chunks).
  3. E = exp(logits + bias) (ScalarE); a [112,112] selector matmul computes
     K*sum_k E broadcast to all 112 (k,h) rows; Rinv = 1/that (DVE);
     en = E * Rinv (DVE). en rows are the normalized conv weights / K.
  4. m[p, k, l] = en[16k + p%16, l+6]: a [112,128] 0/1 selector matmul per k
     replicates 16 head rows across 128 partitions (PE), ScalarE casts to
     bf16. The weight for channel d = 128c + p is row p%16 = d%16, the same
     for every chunk c, so one m tile serves all 8 d-chunks.
  5. conv per d-chunk: acc[:, c, l] = sum_k m_k[:, l] * xtb[:, c, l+k],
     7 bf16 muls + 6 adds split DVE (10 ops) / GPSIMD (3 ops), chunked
     along l so chunk j only depends on s-block j of the front pipeline.
  6. PE-transpose acc back to natural [l, d], ScalarE PSUM->SBUF fp32,
     DMA out. Emission is ordered so per-chunk prep precedes the bulk conv
     ops (engines execute their streams mostly in order).
"""

import numpy as np
import ml_dtypes
from contextlib import ExitStack

import concourse.bacc as bacc
import concourse.tile as tile
from concourse.tile_rust import add_dep_helper
from concourse import mybir
from concourse import bass_utils

K = 7
H = 16
B, S, D = 8, 2048, 1024
L = S - K + 1  # 2042
C = D // 128  # 8 d-chunks
NSB = 4  # s-blocks
SB = S // NSB  # 512
KH = K * H  # 112

F32 = mybir.dt.float32
BF16 = mybir.dt.bfloat16

# byte offsets (per partition) inside the packed constants blob
_OFF_BIAS = 0      # [112, 1] f32
_OFF_IDENT = 4     # [128, 128] f32
_OFF_IDENTB = 516  # [128, 128] bf16
_OFF_SELSUM = 772  # [112, 112] bf16
_OFF_SELK = 996    # [112, 896] bf16
_OFF_WT = 2788     # [128, 8, 112] bf16
_CONST_BYTES = 4580  # 1145 f32 columns


def _host_constants(W, b):
    """Pack bias/ident/identb/selsum/selk/W into one [128, 1145] f32 blob."""
    buf = np.zeros((128, _CONST_BYTES), np.uint8)

    def put(off, arr):
        by = np.ascontiguousarray(arr).view(np.uint8).reshape(arr.shape[0], -1)
        buf[: arr.shape[0], off : off + by.shape[1]] = by

    put(_OFF_BIAS, np.asarray(b, np.float32).reshape(KH, 1))
    put(_OFF_IDENT, np.eye(128, dtype=np.float32))
    put(_OFF_IDENTB, np.eye(128).astype(ml_dtypes.bfloat16))
    h = np.arange(KH) % H
    selsum = ((h[:, None] == h[None, :]) * float(K)).astype(ml_dtypes.bfloat16)
    put(_OFF_SELSUM, selsum)
    selk = np.zeros((KH, K * 128), dtype=ml_dtypes.bfloat16)
    for k in range(K):
        for p in range(128):
            selk[16 * k + p % 16, k * 128 + p] = 1.0
    put(_OFF_SELK, selk)
    # W [D, KH] -> [128, C, KH] chunks (d = c*128 + p)
    wt = np.asarray(W, np.float32).astype(ml_dtypes.bfloat16)
    wt = wt.reshape(C, 128, KH).transpose(1, 0, 2).reshape(128, C * KH)
    put(_OFF_WT, np.ascontiguousarray(wt))
    return buf.view(np.float32)


def build_program():
    nc = bacc.Bacc(
        "TRN2", target_bir_lowering=False, debug=False, enable_asserts=True
    )

    x_d = nc.dram_tensor("x", [S, D], F32, kind="ExternalInput").ap()
    consts_d = nc.dram_tensor(
        "consts", [128, _CONST_BYTES // 4], F32, kind="ExternalInput"
    ).ap()
    out_d = nc.dram_tensor("out", [L, D], F32, kind="ExternalOutput").ap()

    with tile.TileContext(nc) as tc, ExitStack() as ctx:
        singles = ctx.enter_context(tc.tile_pool(name="singles", bufs=1))
        xn_pool = ctx.enter_context(tc.tile_pool(name="xn", bufs=3))
        prodv_pool = ctx.enter_context(tc.tile_pool(name="prodv", bufs=8))
        prodg_pool = ctx.enter_context(tc.tile_pool(name="prodg", bufs=6))
        outn_pool = ctx.enter_context(tc.tile_pool(name="outn", bufs=3))
        m_pool = ctx.enter_context(tc.tile_pool(name="mw", bufs=2))

        p_tp = ctx.enter_context(tc.tile_pool(name="ptp", bufs=2, space="PSUM"))
        p_log = ctx.enter_context(tc.tile_pool(name="plog", bufs=1, space="PSUM"))
        p_sum = ctx.enter_context(tc.tile_pool(name="psumk", bufs=1, space="PSUM"))
        p_mk = ctx.enter_context(tc.tile_pool(name="pmk", bufs=2, space="PSUM"))
        p_otp = ctx.enter_context(tc.tile_pool(name="potp", bufs=2, space="PSUM"))

        # ---- constants: one packed DMA, tiles are views into the blob ----
        cblob = singles.tile([128, _CONST_BYTES // 4], F32)
        nc.sync.dma_start(out=cblob, in_=consts_d)
        cbytes = cblob.bitcast(mybir.dt.uint8)

        def cview(off, nbytes, dt, rows=128):
            return cbytes[:rows, off : off + nbytes].bitcast(dt)

        bias_t = cview(_OFF_BIAS, 4, F32, rows=KH)
        ident_t = cview(_OFF_IDENT, 512, F32)
        identb_t = cview(_OFF_IDENTB, 256, BF16)
        selsum_t = cview(_OFF_SELSUM, 224, BF16, rows=KH)
        selk_t = cview(_OFF_SELK, 1792, BF16, rows=KH).rearrange(
            "c (k p) -> c k p", k=K
        )
        wt = cview(_OFF_WT, 1792, BF16).rearrange("p (c n) -> p c n", c=C)

        # GPSIMD ucode warmup: force the TT library load before real work
        warm = singles.tile([1, 8], BF16)
        nc.gpsimd.tensor_mul(warm, identb_t[:1, :8], identb_t[:1, :8])

        # ---- persistent tensors ----
        xtb = singles.tile([128, C, S], BF16)  # x^T bf16, conv + matmul input
        e_full = singles.tile([KH, S], BF16)  # exp(logits + b)
        rinv = singles.tile([KH, S], F32)  # 1 / (K * sum_k E)
        en = singles.tile([KH, S], BF16)  # normalized kernel weights
        acc_all = singles.tile([128, C, S], BF16)  # conv result, [d, l]

        # ---- emission helpers ----
        xn_tiles = {}

        def load(sb):
            xn = xn_pool.tile([128, 4, D], F32, tag="xn")
            xin = x_d[SB * sb : SB * (sb + 1), :].rearrange(
                "(t p) d -> p t d", p=128
            )
            if sb <= 2:
                # split the first load so front(0) transposes start earlier
                nc.sync.dma_start(out=xn[:, :2, :], in_=xin[:, :2, :])
                nc.sync.dma_start(out=xn[:, 2:, :], in_=xin[:, 2:, :])
            else:
                nc.sync.dma_start(out=xn, in_=xin)
            xn_tiles[sb] = xn

        def front(sb, hold=None, hold_from_c=0):
            """Transpose to [d, s], logits matmul, exp. Transposes (from
            chunk hold_from_c on) ordered after `hold` (a PE instruction) so
            the previous block's softmax-denominator matmul runs first."""
            xn = xn_tiles[sb]
            for c in range(C):
                ptp = p_tp.tile([128, SB], F32, tag="ptp")
                for tt in range(4):
                    tp = nc.tensor.transpose(
                        ptp[:, 128 * tt : 128 * (tt + 1)],
                        xn[:, tt, 128 * c : 128 * (c + 1)],
                        ident_t,
                    )
                    if hold is not None and c >= hold_from_c:
                        add_dep_helper(tp.ins, hold.ins, sync=False,
                                       reason="pe order: front after prev sums")
                nc.scalar.copy(xtb[:, c, SB * sb : SB * (sb + 1)], ptp)
            plog = p_log.tile([KH, SB], F32, tag="plog")
            for c in range(C):
                nc.tensor.matmul(
                    plog,
                    wt[:, c, :],
                    xtb[:, c, SB * sb : SB * (sb + 1)],
                    start=(c == 0),
                    stop=(c == C - 1),
                )
            nc.scalar.activation(
                e_full[:, SB * sb : SB * (sb + 1)],
                plog,
                mybir.ActivationFunctionType.Exp,
                bias=bias_t,
                scale=1.0,
            )

        def denom(sb):
            """softmax denominators + normalized weights for s-block sb."""
            sl = slice(SB * sb, SB * (sb + 1))
            psum = p_sum.tile([KH, SB], F32, tag="psumk")
            mm = nc.tensor.matmul(
                psum, selsum_t, e_full[:, sl], start=True, stop=True
            )
            nc.vector.reciprocal(rinv[:, sl], psum)
            nc.vector.tensor_mul(en[:, sl], e_full[:, sl], rinv[:, sl])
            return mm

        # l-chunk boundaries aligned so prep block j only needs s-block j:
        # mrep(j) reads en[l0+6 : l1+6] = en s-block j; a conv chunk inside
        # [CB[j], CB[j+1]) reads xtb columns only from s-blocks <= j.
        CB = [0, SB - K + 1, 2 * SB - K + 1, 3 * SB - K + 1, L]
        CH = [0, 2 * SB - K + 1, L]  # conv-half boundaries (m tile extents)

        m_tiles = {}

        def mrep(j):
            """m_half[p, k, l-CH[h]] = en[16k + p%16, l + K - 1] for block j."""
            h, off = (j // 2), CB[j] - CH[j // 2]
            if j % 2 == 0:
                mt_new = m_pool.tile([128, K, 2 * SB], BF16, tag="mw")
                m_tiles[h] = mt_new
            mt = m_tiles[h]
            l0, l1 = CB[j], CB[j + 1]
            nl = l1 - l0
            for k in range(K):
                pmk = p_mk.tile([128, SB], F32, tag="pmk")
                nc.tensor.matmul(
                    pmk[:, :nl],
                    selk_t[:, k, :],
                    en[:, l0 + K - 1 : l0 + K - 1 + nl],
                    start=True,
                    stop=True,
                )
                nc.scalar.copy(mt[:, k, off : off + nl], pmk[:, :nl])

        def conv(c, h, l0, l1):
            """acc_all[:, c, l0:l1] = sum_k m_k * x_{+k} (sub-range of half h)."""
            nl = l1 - l0
            off = l0 - CH[h]

            def prod(eng, k, pool, tag):
                p = pool.tile([128, 2 * SB], BF16, tag=tag)
                eng.tensor_mul(
                    p[:, :nl],
                    m_tiles[h][:, k, off : off + nl],
                    xtb[:, c, l0 + k : l0 + k + nl],
                )
                return p

            # Odd k shifts give odd bf16 element offsets into xtb, which
            # break the DVE 2x_1P packed mode (needs 4B-aligned starts) on
            # real HW. GPSIMD is alignment-insensitive, so it takes the odd
            # taps; DVE takes the even taps and the add tree (all product
            # tiles start at column 0, so adds stay aligned).
            p1 = prod(nc.gpsimd, 1, prodg_pool, "prodg")
            p3 = prod(nc.gpsimd, 3, prodg_pool, "prodg")
            p5 = prod(nc.gpsimd, 5, prodg_pool, "prodg")
            # DVE subtree (even taps)
            p0 = prod(nc.vector, 0, prodv_pool, "prodv")
            p2 = prod(nc.vector, 2, prodv_pool, "prodv")
            a02 = prodv_pool.tile([128, 2 * SB], BF16, tag="prodv")
            nc.vector.tensor_add(a02[:, :nl], p0[:, :nl], p2[:, :nl])
            p4 = prod(nc.vector, 4, prodv_pool, "prodv")
            p6 = prod(nc.vector, 6, prodv_pool, "prodv")
            a46 = prodv_pool.tile([128, 2 * SB], BF16, tag="prodv")
            nc.vector.tensor_add(a46[:, :nl], p4[:, :nl], p6[:, :nl])
            a13 = prodv_pool.tile([128, 2 * SB], BF16, tag="prodv")
            nc.vector.tensor_add(a13[:, :nl], p1[:, :nl], p3[:, :nl])
            t0 = prodv_pool.tile([128, 2 * SB], BF16, tag="prodv")
            nc.vector.tensor_add(t0[:, :nl], a02[:, :nl], a46[:, :nl])
            t1 = prodv_pool.tile([128, 2 * SB], BF16, tag="prodv")
            nc.vector.tensor_add(t1[:, :nl], a13[:, :nl], p5[:, :nl])
            nc.vector.tensor_add(
                acc_all[:, c, l0 : l0 + nl], t0[:, :nl], t1[:, :nl]
            )

        def store(lb):
            """transpose acc back to [l, d] and DMA out rows 128*lb..+nl."""
            l0 = 128 * lb
            nl = min(128, L - l0)
            outn = outn_pool.tile([128, D], F32, tag="outn")
            for half in range(2):
                potp = p_otp.tile([128, 512], BF16, tag="potp")
                for cc in range(4):
                    c = 4 * half + cc
                    nc.tensor.transpose(
                        potp[:nl, 128 * cc : 128 * (cc + 1)],
                        acc_all[:, c, l0 : l0 + nl],
                        identb_t,
                    )
                nc.scalar.copy(
                    outn[:nl, 512 * half : 512 * (half + 1)], potp[:nl, :]
                )
            nc.scalar.dma_start(out=out_d[l0 : l0 + nl, :], in_=outn[:nl, :])

        # ---- pipelined emission ----
        # Engines execute their streams mostly in emission order, so all
        # cheap prep for chunk j (denom: DVE recip; mrep: PE+ACT) is emitted
        # before the bulk conv ops that precede its consumers.
        for j in range(4):
            load(j)
        # pair the first two fronts: the first conv chunk needs s-block 0
        # only, the second s-block 1 only.
        front(0)
        prev_sums = denom(0)
        front(1, hold=prev_sums, hold_from_c=4)
        prev_sums = denom(1)
        mrep(0)
        mrep(1)
        front(2, hold=prev_sums)
        prev_sums = denom(2)
        mrep(2)
        for c in range(C):
            conv(c, 0, CB[0], CB[2])
        for lb in range(0, 3):
            store(lb)
        for lb in range(3, 7):
            store(lb)
        front(3, hold=prev_sums)
        denom(3)
        mrep(3)
        for c in range(C):
            conv(c, 1, CB[2], 1792)
        for lb in range(7, 14):
            store(lb)
        for c in range(C):
            conv(c, 1, 1792, CH[2])
        for lb in range(14, 16):
            store(lb)

    nc.compile()
    return nc


_CACHE = {}


def _get_program():
    if "nc" not in _CACHE:
        _CACHE["nc"] = build_program()
    return _CACHE["nc"]


def kernel(x, W, b):
    x = np.asarray(x, dtype=np.float32)
    assert x.shape == (B, S, D), x.shape

    nc = _get_program()
    consts = _host_constants(W, b)
    in_maps = []
    for core in range(B):
        in_maps.append(
            {
                "x": np.ascontiguousarray(x[core]),
                "consts": consts,
            }
        )
    res = bass_utils.run_bass_kernel_spmd(nc, in_maps, core_ids=list(range(B)))
    out = np.stack([res.results[core]["out"] for core in range(B)], axis=0)
    return out



# revision 10
# speedup vs baseline: 1.2286x; 1.2286x over previous
"""Dynamic lightweight convolution TRN2 kernel — banded-matmul design.

out[b,l,d] = (1/K) * sum_k softmax_k(x[b,l+K-1,:] @ W + bias)[k, d%H] * x[b,l+k,d]

B=8, S=2048, D=1024, K=7, H=16, L=S-K+1=2042.
Sharding: data-parallel over batch, one batch element per NeuronCore (8 cores).

Per-core plan — the conv itself runs on the *tensor engine* as banded-matrix
matmuls instead of elementwise DVE/GPSIMD work (which bottlenecked the old
design at ~104us busy per engine):

  1. x is loaded by GPSIMD (SWDGE) casting DMAs: f32 HBM -> bf16 SBUF chunks
     xb[i] [128, 1024] in natural [s, d] layout (halves input DMA bytes and
     removes the cast pass entirely).
  2. Logits path (as before, but from xb): PE-transpose xb -> xT per s-block,
     logits = W^T @ xT (PE, fp32 psum), e = exp(logits + bias) (ACT),
     denominators via a [112,112] selector matmul (PE), rinv = 1/. (DVE),
     en = e * rinv (DVE)  — en[16k+h, l+6] is the normalized tap weight.
  3. Shifted/regrouped copies build Et[16j+h, s] = en[16(6-j)+h, s+j]
     (j = 6-k), then PE-transposes give T[s, r] (r = 16j+h), stored in
     T_all [128, chunk, 112].
  4. Band construction via a DRAM bounce (SBUF scatter DMAs cannot skew more
     than 128 bytes across partitions — hw descriptor field limit — but DRAM
     strides are free): T_all[:, b] is written to a zero-filled DRAM image at
     skewed offsets IMG_SKEW*p + r, and read back with row pitch IMG_PITCH,
     which lands T[p, r] at band position (p, 16p + r).  Non-band cells stay
     zero across blocks since each block overwrites exactly the same cells.
  5. Conv for 128-row output block b: for each h, a banded matmul
       out[l, d'] = sum_s A1_h[s, l] * xb[b][s, 16d'+h]   (+ A2_h tail rows)
     with stationary A1_h = a1[:, h : h+2048 : 16] (the h-interleaved band
     view) and 6 extra contraction rows from xb[b+1] handling the s-window
     straddle.  PE cost is only out-cols * 1 cyc/row; LdWeights are free.
  6. psum [128, 1024] (h-major) -> SBUF with a de-interleaving ACT copy
     (dst AP reorders 64h+d' -> 16d'+h), then DMA rows to HBM.
"""

import numpy as np
import ml_dtypes
from contextlib import ExitStack

import concourse.bacc as bacc
import concourse.tile as tile
from concourse import mybir
from concourse import bass_utils
from concourse.ap import AP

K = 7
H = 16
B, S, D = 8, 2048, 1024
L = S - K + 1  # 2042
C = D // 128  # 8 d-chunks
NCH = S // 128  # 16 s-chunks
NB = 16  # output blocks of 128 rows (last has 122 valid)
KH = K * H  # 112

SLOT0 = 96  # img col of (l_rel=0, h=0): band tiles are loaded from this col
ACOLS = 2064  # band-tile cols actually needed by the stationary views
IMG_PITCH = 2256  # image read pitch (elements)
IMG_SKEW = IMG_PITCH + 16  # image write pitch: +16 elems (one slot) per row
IMG1_ELEMS = IMG_PITCH * 128
IMG2_ELEMS = IMG_PITCH * 6

F32 = mybir.dt.float32
BF16 = mybir.dt.bfloat16

# byte offsets (per partition) inside the packed constants blob
_OFF_BIAS = 0      # [112, 1] f32
_OFF_IDENTB = 4    # [128, 128] bf16
_OFF_SELSUM = 260  # [112, 112] bf16
_OFF_WT = 484      # [128, 8, 112] bf16
_CONST_BYTES = 2276  # 569 f32 columns


def _host_constants(W, b):
    """Pack bias/identb/selsum/W into one [128, 569] f32 blob."""
    buf = np.zeros((128, _CONST_BYTES), np.uint8)

    def put(off, arr):
        by = np.ascontiguousarray(arr).view(np.uint8).reshape(arr.shape[0], -1)
        buf[: arr.shape[0], off : off + by.shape[1]] = by

    # Permute the k-axis (k -> 6-k) of W and bias so that logits/e/en rows
    # come out in j-order (row 16j+h is the weight for tap k=6-j), matching
    # the band-image run layout r = 16j+h.
    perm = np.array([16 * (K - 1 - j) + h for j in range(K) for h in range(H)])
    put(_OFF_BIAS, np.asarray(b, np.float32)[perm].reshape(KH, 1))
    put(_OFF_IDENTB, np.eye(128).astype(ml_dtypes.bfloat16))
    hh = np.arange(KH) % H
    selsum = ((hh[:, None] == hh[None, :]) * float(K)).astype(ml_dtypes.bfloat16)
    put(_OFF_SELSUM, selsum)
    # W [D, KH] -> permuted -> [128, C, KH] chunks (d = c*128 + p)
    wt = np.asarray(W, np.float32)[:, perm].astype(ml_dtypes.bfloat16)
    wt = wt.reshape(C, 128, KH).transpose(1, 0, 2).reshape(128, C * KH)
    put(_OFF_WT, np.ascontiguousarray(wt))
    return buf.view(np.float32)


def build_program():
    nc = bacc.Bacc(
        "TRN2", target_bir_lowering=False, debug=False, enable_asserts=True
    )

    x_d = nc.dram_tensor("x", [S, D], F32, kind="ExternalInput").ap()
    consts_d = nc.dram_tensor(
        "consts", [128, _CONST_BYTES // 4], F32, kind="ExternalInput"
    ).ap()
    out_d = nc.dram_tensor("out", [L, D], F32, kind="ExternalOutput").ap()
    img1 = [
        nc.dram_tensor(f"img1{i}", [IMG1_ELEMS], BF16, kind="Internal").ap()
        for i in range(2)
    ]
    img2 = [
        nc.dram_tensor(f"img2{i}", [IMG2_ELEMS], BF16, kind="Internal").ap()
        for i in range(2)
    ]

    with tile.TileContext(nc) as tc, ExitStack() as ctx:
        singles = ctx.enter_context(tc.tile_pool(name="singles", bufs=1))
        xT_pool = ctx.enter_context(tc.tile_pool(name="xT", bufs=2))
        a1_pool = ctx.enter_context(tc.tile_pool(name="a1", bufs=2))
        a2_pool = ctx.enter_context(tc.tile_pool(name="a2", bufs=2))
        outs_pool = ctx.enter_context(tc.tile_pool(name="outs", bufs=3))

        p_tp = ctx.enter_context(tc.tile_pool(name="ptp", bufs=1, space="PSUM"))
        p_log = ctx.enter_context(tc.tile_pool(name="plog", bufs=1, space="PSUM"))
        p_sd = ctx.enter_context(tc.tile_pool(name="psd", bufs=1, space="PSUM"))
        p_t = ctx.enter_context(tc.tile_pool(name="pt", bufs=1, space="PSUM"))
        p_out = ctx.enter_context(tc.tile_pool(name="pout", bufs=2, space="PSUM"))

        # ---- constants: one packed DMA, tiles are views into the blob ----
        cblob = singles.tile([128, _CONST_BYTES // 4], F32)
        nc.sync.dma_start(out=cblob, in_=consts_d)
        cbytes = cblob.bitcast(mybir.dt.uint8)

        def cview(off, nbytes, dt, rows=128):
            return cbytes[:rows, off : off + nbytes].bitcast(dt)

        bias_t = cview(_OFF_BIAS, 4, F32, rows=KH)
        identb_t = cview(_OFF_IDENTB, 256, BF16)
        selsum_t = cview(_OFF_SELSUM, 224, BF16, rows=KH)
        wt = cview(_OFF_WT, 1792, BF16).rearrange("p (c n) -> p c n", c=C)

        # GPSIMD ucode warmup
        warm = singles.tile([1, 8], BF16)
        nc.gpsimd.tensor_mul(warm, identb_t[:1, :8], identb_t[:1, :8])

        # ---- persistent tensors ----
        xb = [
            singles.tile([128, D], BF16, name=f"xb{i}") for i in range(NCH)
        ]
        e_full = singles.tile([KH, S], BF16)
        rinv = singles.tile([KH, S], F32)
        en = singles.tile([KH, S], BF16)
        et = singles.tile([KH, S], BF16)  # et[16j+h, s] = en[16j+h, s+j]
        t_all = singles.tile([128, NCH, KH], BF16)  # T[s, r], chunked
        zt = singles.tile([128, IMG_PITCH], BF16)  # zeros for image fill

        # ---- prologue ----
        nc.vector.memset(zt, 0.0)
        # et tail cols: only read for invalid outputs l >= L; keep finite
        nc.vector.memset(et[:, S - 6 :], 0.0)
        for i in range(2):
            nc.sync.dma_start(
                out=AP(tensor=img1[i].tensor, offset=0,
                       ap=[[IMG_PITCH, 128], [1, IMG_PITCH]]),
                in_=zt[:, :],
            )
            nc.sync.dma_start(
                out=AP(tensor=img2[i].tensor, offset=0,
                       ap=[[IMG_PITCH, 6], [1, IMG_PITCH]]),
                in_=zt[:6, :],
            )
        # casting input DMAs (f32 HBM -> bf16 SBUF) via GPSIMD SWDGE
        for i in range(NCH):
            nc.gpsimd.dma_start(out=xb[i], in_=x_d[128 * i : 128 * (i + 1), :])

        # ---- stage helpers ----
        def front(sb):
            """Transpose chunks 4sb..4sb+3, logits, exp, denom, rinv, en."""
            sl = slice(512 * sb, 512 * (sb + 1))
            xTt = xT_pool.tile([128, C, 512], BF16, tag="xT")
            for q in range(4):
                i = 4 * sb + q
                ptp = p_tp.tile([128, D], BF16, tag="ptp")
                for c in range(C):
                    nc.tensor.transpose(
                        ptp[:, 128 * c : 128 * (c + 1)],
                        xb[i][:, 128 * c : 128 * (c + 1)],
                        identb_t,
                    )
                nc.vector.tensor_copy(
                    xTt[:, :, 128 * q : 128 * (q + 1)],
                    ptp.rearrange("p (c s) -> p c s", c=C),
                )
            plog = p_log.tile([KH, 512], F32, tag="plog")
            for c in range(C):
                nc.tensor.matmul(
                    plog, wt[:, c, :], xTt[:, c, :],
                    start=(c == 0), stop=(c == C - 1),
                )
            nc.scalar.activation(
                e_full[:, sl], plog,
                mybir.ActivationFunctionType.Exp, bias=bias_t, scale=1.0,
            )
            psd = p_sd.tile([KH, 512], F32, tag="psd")
            nc.tensor.matmul(psd, selsum_t, e_full[:, sl], start=True, stop=True)
            nc.vector.reciprocal(rinv[:, sl], psd)
            nc.vector.tensor_mul(en[:KH, sl], e_full[:, sl], rinv[:, sl])

        def shifts(half):
            """et[16j+h, s] = en[16j+h, s+j] for s in one half of the sequence.

            Engine copies can't start at partition 16j (BIR rule: starts must
            be 0/32/64/96) and SBUF DMA APs need pitch-exact partition steps,
            so this is one plain 2-dim SBUF->SBUF DMA per j-group.
            """
            c0 = 1024 * half
            for j in range(K):
                ln = 1024 if half == 0 else 1024 - j
                nc.sync.dma_start(
                    out=AP(tensor=et[:, :].tensor, offset=16 * j * S + c0,
                           ap=[[S, 16], [1, ln]]),
                    in_=AP(tensor=en[:, :].tensor, offset=16 * j * S + c0 + j,
                           ap=[[S, 16], [1, ln]]),
                )

        def t_chunks(lo, hi):
            for i in range(lo, hi):
                pt = p_t.tile([128, KH], BF16, tag="pt")
                nc.tensor.transpose(
                    pt, et[:, 128 * i : 128 * (i + 1)], identb_t[:KH, :KH]
                )
                nc.vector.tensor_copy(t_all[:, i, :], pt)

        def block(b):
            """Banded conv for output rows 128b .. 128b+nl."""
            i1, i2 = img1[b % 2], img2[b % 2]
            nc.sync.dma_start(
                out=AP(tensor=i1.tensor, offset=0,
                       ap=[[IMG_SKEW, 128], [1, KH]]),
                in_=t_all[:, b, :],
            )
            if b < NB - 1:
                nc.sync.dma_start(
                    out=AP(tensor=i2.tensor, offset=2048,
                           ap=[[IMG_SKEW, 6], [1, KH]]),
                    in_=t_all[:6, b + 1, :],
                )
            a1 = a1_pool.tile([128, ACOLS], BF16, tag="a1")
            nc.sync.dma_start(
                out=a1,
                in_=AP(tensor=i1.tensor, offset=SLOT0,
                       ap=[[IMG_PITCH, 128], [1, ACOLS]]),
            )
            if b < NB - 1:
                a2 = a2_pool.tile([6, ACOLS], BF16, tag="a2")
                nc.sync.dma_start(
                    out=a2,
                    in_=AP(tensor=i2.tensor, offset=SLOT0,
                           ap=[[IMG_PITCH, 6], [1, ACOLS]]),
                )
            po = p_out.tile([128, D], F32, tag="pout")
            for h in range(H):
                stat1 = a1[:, h : h + 16 * 128 : 16]
                nc.tensor.matmul(
                    po[:, 64 * h : 64 * (h + 1)], stat1,
                    xb[b][:, h :: H],
                    start=True, stop=(b == NB - 1),
                )
                if b < NB - 1:
                    stat2 = a2[:, h : h + 16 * 128 : 16]
                    nc.tensor.matmul(
                        po[:, 64 * h : 64 * (h + 1)], stat2,
                        xb[b + 1][:6, h :: H],
                        start=False, stop=True,
                    )
            ob = outs_pool.tile([128, D], F32, tag="outs")
            nc.scalar.copy(
                ob.rearrange("p (dp h) -> p h dp", h=H),
                po.rearrange("p (h dp) -> p h dp", h=H),
            )
            nl = min(128, L - 128 * b)
            nc.scalar.dma_start(
                out=out_d[128 * b : 128 * b + nl, :], in_=ob[:nl, :]
            )

        # ---- pipelined emission ----
        front(0)
        front(1)
        front(2)
        shifts(0)  # needs en cols [0, 1024+6) -> after front(2)
        t_chunks(0, 8)
        for b in range(0, 7):
            block(b)
        front(3)
        shifts(1)
        t_chunks(8, 16)
        for b in range(7, 16):
            block(b)

    nc.compile()
    return nc


_CACHE = {}


def _get_program():
    if "nc" not in _CACHE:
        _CACHE["nc"] = build_program()
    return _CACHE["nc"]


def kernel(x, W, b):
    x = np.asarray(x, dtype=np.float32)
    assert x.shape == (B, S, D), x.shape

    nc = _get_program()
    consts = _host_constants(W, b)
    in_maps = []
    for core in range(B):
        in_maps.append(
            {
                "x": np.ascontiguousarray(x[core]),
                "consts": consts,
            }
        )
    res = bass_utils.run_bass_kernel_spmd(nc, in_maps, core_ids=list(range(B)))
    out = np.stack([res.results[core]["out"] for core in range(B)], axis=0)
    return out


# revision 11
# speedup vs baseline: 1.2664x; 1.0308x over previous
"""Dynamic lightweight convolution TRN2 kernel — banded-matmul design.

out[b,l,d] = (1/K) * sum_k softmax_k(x[b,l+K-1,:] @ W + bias)[k, d%H] * x[b,l+k,d]

B=8, S=2048, D=1024, K=7, H=16, L=S-K+1=2042.
Sharding: data-parallel over batch, one batch element per NeuronCore (8 cores).

Per-core plan — the conv itself runs on the *tensor engine* as banded-matrix
matmuls instead of elementwise DVE/GPSIMD work (which bottlenecked the old
design at ~104us busy per engine):

  1. x is loaded by GPSIMD (SWDGE) casting DMAs: f32 HBM -> bf16 SBUF chunks
     xb[i] [128, 1024] in natural [s, d] layout (halves input DMA bytes and
     removes the cast pass entirely).
  2. Logits path (from xb): PE-transpose xb -> xT per s-block, logits =
     W^T @ xT (PE, fp32 psum), e = exp(logits + bias) (ACT), denominators via
     a [112,112] selector matmul (PE), rinv = 1/. (DVE), en = e * rinv (DVE).
     W/bias columns are host-permuted k -> 6-k, so row 16j+h of en is the
     normalized weight of tap k = 6-j.
  3. Per-j-group shifted SBUF->SBUF DMAs build et[16j+h, s] = en[16j+h, s+j]
     (engine copies can't start at partition 16j, DMA can), then
     PE-transposes give T[s, r] (r = 16j+h), stored in T_all [128, chunk, 112].
  4. Band construction via a DRAM bounce (SBUF scatter DMAs cannot skew more
     than 128 bytes across partitions — hw descriptor drift limit — but DRAM
     strides are free): T_all[:, b] is written to a zero-filled DRAM image at
     skewed offsets IMG_SKEW*p + r and read back with row pitch IMG_PITCH,
     which lands T[p, r] at band position (p, 16p + r).  Non-band cells stay
     zero across blocks since each block overwrites exactly the same cells.
  5. Conv for 128-row output block b: for each h, a banded matmul
       out[l, d'] = sum_s A1_h[s, l] * xb[b][s, 16d'+h]
     with stationary A1_h = a1[:, h : h+2048 : 16] (h-interleaved band view).
     The 6-row contraction tail (s in the next chunk) uses a2: its band cells
     are exactly the *left guard* cells of img1(b+1), so a tiny [6, 112] load
     from img1(b+1) (rest of a2 is memset zero once) + a second matmul
     accumulating into the same psum.  PE cost is out-cols * 1 cyc/row only;
     LdWeights are free.
  6. psum [128, 1024] (h-major) -> SBUF staging with a de-interleaving copy
     (dst AP reorders 64h+d' -> 16d'+h); two blocks share one staging tile
     and one paired store DMA (3-dim DRAM dst AP).
"""

import numpy as np
import ml_dtypes
from contextlib import ExitStack

import concourse.bacc as bacc
import concourse.tile as tile
from concourse import mybir
from concourse import bass_utils
from concourse.ap import AP

K = 7
H = 16
B, S, D = 8, 2048, 1024
L = S - K + 1  # 2042
C = D // 128  # 8 d-chunks
NCH = S // 128  # 16 s-chunks
NB = 16  # output blocks of 128 rows (last has 122 valid)
KH = K * H  # 112

SLOT0 = 96  # img col of (l_rel=0, h=0): band tiles are loaded from this col
ACOLS = 2064  # band-tile cols needed by the stationary views
A2LO = 1952  # a2 col of (l_rel=122, h=0); cols below are zero
IMG_PITCH = 2256  # image read pitch (elements)
IMG_SKEW = IMG_PITCH + 16  # image write pitch: +16 elems (one slot) per row
IMG_ELEMS = IMG_PITCH * 128

F32 = mybir.dt.float32
BF16 = mybir.dt.bfloat16

# byte offsets (per partition) inside the packed constants blob
_OFF_BIAS = 0      # [112, 1] f32
_OFF_IDENTB = 4    # [128, 128] bf16
_OFF_SELSUM = 260  # [112, 112] bf16
_OFF_WT = 484      # [128, 8, 112] bf16
_CONST_BYTES = 2276  # 569 f32 columns


def _host_constants(W, b):
    """Pack bias/identb/selsum/W into one [128, 569] f32 blob."""
    buf = np.zeros((128, _CONST_BYTES), np.uint8)

    def put(off, arr):
        by = np.ascontiguousarray(arr).view(np.uint8).reshape(arr.shape[0], -1)
        buf[: arr.shape[0], off : off + by.shape[1]] = by

    # Permute the k-axis (k -> 6-k) of W and bias so that logits/e/en rows
    # come out in j-order (row 16j+h is the weight for tap k=6-j), matching
    # the band-image run layout r = 16j+h.
    perm = np.array([16 * (K - 1 - j) + h for j in range(K) for h in range(H)])
    put(_OFF_BIAS, np.asarray(b, np.float32)[perm].reshape(KH, 1))
    put(_OFF_IDENTB, np.eye(128).astype(ml_dtypes.bfloat16))
    hh = np.arange(KH) % H
    selsum = ((hh[:, None] == hh[None, :]) * float(K)).astype(ml_dtypes.bfloat16)
    put(_OFF_SELSUM, selsum)
    # W [D, KH] -> permuted -> [128, C, KH] chunks (d = c*128 + p)
    wt = np.asarray(W, np.float32)[:, perm].astype(ml_dtypes.bfloat16)
    wt = wt.reshape(C, 128, KH).transpose(1, 0, 2).reshape(128, C * KH)
    put(_OFF_WT, np.ascontiguousarray(wt))
    return buf.view(np.float32)


def build_program():
    nc = bacc.Bacc(
        "TRN2", target_bir_lowering=False, debug=False, enable_asserts=True
    )

    x_d = nc.dram_tensor("x", [S, D], F32, kind="ExternalInput").ap()
    consts_d = nc.dram_tensor(
        "consts", [128, _CONST_BYTES // 4], F32, kind="ExternalInput"
    ).ap()
    out_d = nc.dram_tensor("out", [L, D], F32, kind="ExternalOutput").ap()
    img1 = [
        nc.dram_tensor(f"img1{i}", [IMG_ELEMS], BF16, kind="Internal").ap()
        for i in range(2)
    ]

    with tile.TileContext(nc) as tc, ExitStack() as ctx:
        singles = ctx.enter_context(tc.tile_pool(name="singles", bufs=1))
        xT_pool = ctx.enter_context(tc.tile_pool(name="xT", bufs=2))
        a1_pool = ctx.enter_context(tc.tile_pool(name="a1", bufs=2))
        outs_pool = ctx.enter_context(tc.tile_pool(name="outs", bufs=2))

        p_tp = ctx.enter_context(tc.tile_pool(name="ptp", bufs=2, space="PSUM"))
        p_log = ctx.enter_context(tc.tile_pool(name="plog", bufs=1, space="PSUM"))
        p_sd = ctx.enter_context(tc.tile_pool(name="psd", bufs=1, space="PSUM"))
        p_out = ctx.enter_context(tc.tile_pool(name="pout", bufs=2, space="PSUM"))

        # ---- constants: one packed DMA, tiles are views into the blob ----
        cblob = singles.tile([128, _CONST_BYTES // 4], F32)
        nc.sync.dma_start(out=cblob, in_=consts_d)
        cbytes = cblob.bitcast(mybir.dt.uint8)

        def cview(off, nbytes, dt, rows=128):
            return cbytes[:rows, off : off + nbytes].bitcast(dt)

        bias_t = cview(_OFF_BIAS, 4, F32, rows=KH)
        identb_t = cview(_OFF_IDENTB, 256, BF16)
        selsum_t = cview(_OFF_SELSUM, 224, BF16, rows=KH)
        wt = cview(_OFF_WT, 1792, BF16).rearrange("p (c n) -> p c n", c=C)

        # GPSIMD ucode warmup
        warm = singles.tile([1, 8], BF16)
        nc.gpsimd.tensor_mul(warm, identb_t[:1, :8], identb_t[:1, :8])

        # ---- persistent tensors ----
        xb = [
            singles.tile([128, D], BF16, name=f"xb{i}") for i in range(NCH)
        ]
        e_full = singles.tile([KH, S], BF16)
        rinv = singles.tile([KH, S], F32)
        en = singles.tile([KH, S], BF16)
        et = singles.tile([KH, S], BF16)  # et[16j+h, s] = en[16j+h, s+j]
        t_all = singles.tile([128, NCH, KH], BF16)  # T[s, r], chunked
        zt = singles.tile([128, IMG_PITCH], BF16)  # zeros for image fill
        a2t = [
            singles.tile([6, ACOLS], BF16, name=f"a2t{i}") for i in range(2)
        ]

        # ---- prologue ----
        nc.vector.memset(zt, 0.0)
        # et tail cols: only read for invalid outputs l >= L; keep finite
        nc.vector.memset(et[:, S - 6 :], 0.0)
        # a2 tiles: cols < A2LO are always zero (out-of-band)
        nc.vector.memset(a2t[0], 0.0)
        nc.vector.memset(a2t[1], 0.0)
        for i in range(2):
            nc.sync.dma_start(
                out=AP(tensor=img1[i].tensor, offset=0,
                       ap=[[IMG_PITCH, 128], [1, IMG_PITCH]]),
                in_=zt[:, :],
            )
        # casting input DMAs (f32 HBM -> bf16 SBUF) via GPSIMD SWDGE
        for i in range(NCH):
            nc.gpsimd.dma_start(out=xb[i], in_=x_d[128 * i : 128 * (i + 1), :])

        # ---- stage helpers ----
        def front(sb):
            """Transpose chunks 4sb..4sb+3, logits, exp, denom, rinv, en."""
            sl = slice(512 * sb, 512 * (sb + 1))
            xTt = xT_pool.tile([128, C, 512], BF16, tag="xT")
            for q in range(4):
                i = 4 * sb + q
                ptp = p_tp.tile([128, D], BF16, tag="ptp")
                for c in range(C):
                    nc.tensor.transpose(
                        ptp[:, 128 * c : 128 * (c + 1)],
                        xb[i][:, 128 * c : 128 * (c + 1)],
                        identb_t,
                    )
                eng = nc.vector if q % 2 == 0 else nc.scalar
                cp = (eng.tensor_copy if q % 2 == 0 else eng.copy)
                cp(
                    xTt[:, :, 128 * q : 128 * (q + 1)],
                    ptp.rearrange("p (c s) -> p c s", c=C),
                )
            plog = p_log.tile([KH, 512], F32, tag="plog")
            for c in range(C):
                nc.tensor.matmul(
                    plog, wt[:, c, :], xTt[:, c, :],
                    start=(c == 0), stop=(c == C - 1),
                )
            nc.scalar.activation(
                e_full[:, sl], plog,
                mybir.ActivationFunctionType.Exp, bias=bias_t, scale=1.0,
            )
            psd = p_sd.tile([KH, 512], F32, tag="psd")
            nc.tensor.matmul(psd, selsum_t, e_full[:, sl], start=True, stop=True)
            nc.vector.reciprocal(rinv[:, sl], psd)
            nc.vector.tensor_mul(en[:, sl], e_full[:, sl], rinv[:, sl])

        def shifts(sb):
            """et[16j+h, s] = en[16j+h, s+j] for s-block sb — one DMA per j.

            Engine copies can't start at partition 16j (BIR rule: starts must
            be 0/32/64/96) and SBUF DMA APs need pitch-exact partition steps,
            so: plain 2-dim SBUF->SBUF DMAs, one per j-group.
            """
            c0 = 512 * sb
            for j in range(K):
                ln = 512 if sb < 3 else 512 - j
                nc.sync.dma_start(
                    out=AP(tensor=et[:, :].tensor, offset=16 * j * S + c0,
                           ap=[[S, 16], [1, ln]]),
                    in_=AP(tensor=en[:, :].tensor, offset=16 * j * S + c0 + j,
                           ap=[[S, 16], [1, ln]]),
                )

        def t_chunks(lo, hi):
            for i in range(lo, hi):
                pt = p_tp.tile([128, D], BF16, tag="ptp")
                nc.tensor.transpose(
                    pt[:, :KH], et[:, 128 * i : 128 * (i + 1)],
                    identb_t[:KH, :KH],
                )
                nc.vector.tensor_copy(t_all[:, i, :], pt[:, :KH])

        def dma1(b):
            """T chunk b -> band image (skewed write; DRAM strides are free)."""
            nc.sync.dma_start(
                out=AP(tensor=img1[b % 2].tensor, offset=0,
                       ap=[[IMG_SKEW, 128], [1, KH]]),
                in_=t_all[:, b, :],
            )

        def block(b, ob, obhalf):
            """Banded conv for output rows 128b .. 128b+nl -> staging tile."""
            if b + 1 < NB:
                dma1(b + 1)
                # a2 tail: the left-guard cells of img1(b+1)
                nc.gpsimd.dma_start(
                    out=a2t[b % 2][:, A2LO : A2LO + KH],
                    in_=AP(tensor=img1[(b + 1) % 2].tensor, offset=0,
                           ap=[[IMG_PITCH, 6], [1, KH]]),
                )
            a1 = a1_pool.tile([128, ACOLS], BF16, tag="a1")
            nc.sync.dma_start(
                out=a1,
                in_=AP(tensor=img1[b % 2].tensor, offset=SLOT0,
                       ap=[[IMG_PITCH, 128], [1, ACOLS]]),
            )
            po = p_out.tile([128, D], F32, tag="pout")
            for h in range(H):
                stat1 = a1[:, h : h + 16 * 128 : 16]
                nc.tensor.matmul(
                    po[:, 64 * h : 64 * (h + 1)], stat1,
                    xb[b][:, h :: H],
                    start=True, stop=(b == NB - 1),
                )
                if b + 1 < NB:
                    stat2 = a2t[b % 2][:, h : h + 16 * 128 : 16]
                    nc.tensor.matmul(
                        po[:, 64 * h : 64 * (h + 1)], stat2,
                        xb[b + 1][:6, h :: H],
                        start=False, stop=True,
                    )
            # de-interleave h-major psum into natural channel order
            eng_copy = nc.scalar.copy if b % 2 == 0 else nc.vector.tensor_copy
            eng_copy(
                ob[:, 1024 * obhalf : 1024 * (obhalf + 1)].rearrange(
                    "p (dp h) -> p h dp", h=H
                ),
                po.rearrange("p (h dp) -> p h dp", h=H),
            )

        def run_pair(q):
            """Blocks 2q, 2q+1 -> one staging tile -> one (or two) stores."""
            ob = outs_pool.tile([128, 2 * D], F32, tag="outs")
            block(2 * q, ob, 0)
            block(2 * q + 1, ob, 1)
            r0 = 256 * q
            if q < 7:
                nc.scalar.dma_start(
                    out=AP(tensor=out_d.tensor, offset=r0 * D,
                           ap=[[D, 128], [128 * D, 2], [1, D]]),
                    in_=AP(tensor=ob[:, :].tensor, offset=0,
                           ap=[[2 * D, 128], [D, 2], [1, D]]),
                )
            else:
                nc.scalar.dma_start(
                    out=out_d[r0 : r0 + 128, :], in_=ob[:, :D]
                )
                nc.scalar.dma_start(
                    out=out_d[r0 + 128 : L, :], in_=ob[: L - r0 - 128, D:]
                )

        # ---- pipelined emission ----
        front(0)
        front(1)
        shifts(0)  # needs en cols [0, 512+6) -> after front(1)
        t_chunks(0, 4)
        dma1(0)
        front(2)
        shifts(1)
        t_chunks(4, 8)
        run_pair(0)  # blocks 0,1
        run_pair(1)  # blocks 2,3
        front(3)
        shifts(2)
        t_chunks(8, 12)
        run_pair(2)
        run_pair(3)  # block 7 emits dma1(8): needs T chunk 8 ✓
        shifts(3)
        t_chunks(12, 16)
        for q in range(4, 8):
            run_pair(q)

    nc.compile()
    return nc


_CACHE = {}


def _get_program():
    if "nc" not in _CACHE:
        _CACHE["nc"] = build_program()
    return _CACHE["nc"]


def kernel(x, W, b):
    x = np.asarray(x, dtype=np.float32)
    assert x.shape == (B, S, D), x.shape

    nc = _get_program()
    consts = _host_constants(W, b)
    in_maps = []
    for core in range(B):
        in_maps.append(
            {
                "x": np.ascontiguousarray(x[core]),
                "consts": consts,
            }
        )
    res = bass_utils.run_bass_kernel_spmd(nc, in_maps, core_ids=list(range(B)))
    out = np.stack([res.results[core]["out"] for core in range(B)], axis=0)
    return out
